# revision 1
# baseline (speedup 1.0000x reference)
"""Trainium2 Bass kernel for a dense transformer decoder block.

Sharding: sequence-parallel over the 4096 (B*T) rows -> 8 cores, 512 rows
each (batch = core//4, row block = core%4). No collectives: each core
recomputes full-batch self K/V and cross K/V (uniform SPMD program); the
causal structure is handled with a host-side row permutation (own rows
last) + per-partition exp bias (0 / -30) + a small triangular mask on the
4 diagonal s-tiles.

Layout: all activations live transposed [C(part-tiles), tokens(free)] so
every projection is lhsT=W (stationary), rhs=activation — no on-device
transposes. Softmax runs on scores^T [s, q]: exp on ScalarE with fused
1/sqrt(d) scale + per-partition mask bias; denominator via an appended
ones-column on V (row 64 of the PV accumulator = sum of probs).
Matmul operands are bf16, accumulation/residual/LN stats in fp32.
"""

import sys
import numpy as np

sys.path.insert(0, "/opt/trn_rl_repo")

import ml_dtypes  # noqa: E402
import concourse.bass as bass  # noqa: E402
import concourse.bacc as bacc  # noqa: E402
import concourse.tile as tile  # noqa: E402
from concourse import mybir  # noqa: E402
from concourse.bass_utils import run_bass_kernel_spmd  # noqa: E402

BF = ml_dtypes.bfloat16
F32 = mybir.dt.float32
BF16 = mybir.dt.bfloat16
AF = mybir.ActivationFunctionType
ALU = mybir.AluOpType

B, T, SE, C, H, HS = 2, 2048, 2048, 1024, 16, 64
NCORE = 8
RB = 512          # rows per core
KT = C // 128     # 8 k-tiles over C
ST = T // 128     # 16 s-tiles
EPS = 1e-5
NEG = -30.0
P = 128


def _build_nc():
    nc = bacc.Bacc(None, target_bir_lowering=False)

    def din(name, shape, dt=BF16):
        return nc.dram_tensor(name, shape, dt, kind="ExternalInput").ap()

    d = {}
    d["xT"] = din("xT", [C, T], BF16)         # permuted x^T (own rows last)
    d["xeT"] = din("xeT", [C, SE], BF16)      # x_e^T
    for n in ["wsq", "wsk", "wsv", "wcq", "wck", "wcv", "wmp", "wcp"]:
        d[n] = din(n, [C, C])
    d["wf1"] = din("wf1", [C, 4 * C])
    d["wf2"] = din("wf2", [4 * C, C])
    for n in ["bsq", "bsk", "bsv", "bcq", "bck", "bcv", "bmp", "bcp", "bf2",
              "g1", "t1", "g2", "t2", "g3", "t3"]:
        d[n] = din(n, [C], F32)
    d["bf1"] = din("bf1", [4 * C], F32)
    d["sbias"] = din("sbias", [T], F32)
    d["smask"] = din("smask", [RB, RB], BF16)
    d["outT"] = nc.dram_tensor("outT", [C, RB], F32, kind="ExternalOutput").ap()

    with tile.TileContext(nc) as tc:
        _emit(tc, nc, d)
    nc.finalize()
    return nc


def _emit(tc, nc, d):
    from contextlib import ExitStack
    ctx = ExitStack()
    ctx.enter_context(nc.allow_low_precision(reason="bf16 matmul operands"))

    # ---------------- persistent pools ----------------
    consts = ctx.enter_context(tc.tile_pool(name="consts", bufs=1))
    respool = ctx.enter_context(tc.tile_pool(name="respool", bufs=2))
    xnpool = ctx.enter_context(tc.tile_pool(name="xnpool", bufs=1))
    epool = ctx.enter_context(tc.tile_pool(name="epool", bufs=2))
    lnsmall = ctx.enter_context(tc.tile_pool(name="lnsmall", bufs=2))

    ones = consts.tile([P, P], BF16)
    nc.vector.memset(ones, 1.0)
    sb_sbias = consts.tile([P, ST], F32)
    nc.sync.dma_start(out=sb_sbias, in_=d["sbias"].rearrange("(st p) -> p st", p=P))
    sb_smask = consts.tile([P, 4, RB], BF16)
    smr = d["smask"].rearrange("(i p) q -> p i q", p=P)
    for i in range(4):
        nc.sync.dma_start(out=sb_smask[:, i, :], in_=smr[:, i, :])

    def colvec(name1d, n=KT):
        t = consts.tile([P, n], F32, tag=f"cv_{name1d}")
        nc.sync.dma_start(out=t, in_=d[name1d].rearrange("(m p) -> p m", p=P))
        return t

    cv = {n: colvec(n) for n in ["bsq", "bsk", "bsv", "bcq", "bck", "bcv",
                                 "bmp", "bcp", "bf2", "g1", "t1", "g2", "t2",
                                 "g3", "t3"]}
    cv["bf1"] = colvec("bf1", 32)

    def ln_apply(pools, src, src_is_bf, xn_out, g, b):
        """LayerNorm over C for RB token columns. src [P, KT, RB];
        writes xn_out [P, KT, RB] bf16."""
        pstat, pbc, sbtmp = pools
        s1 = pstat.tile([1, RB], F32, tag="s1")
        s2 = pstat.tile([1, RB], F32, tag="s1")
        for k in range(KT):
            if src_is_bf:
                xbk = src[:, k, :]
            else:
                xbk = epool.tile([P, RB], BF16, tag="xbk")
                nc.vector.tensor_copy(out=xbk, in_=src[:, k, :])
            xsqk = epool.tile([P, RB], BF16, tag="xsqk")
            nc.vector.tensor_mul(xsqk, xbk, xbk)
            nc.tensor.matmul(s1, ones[:, 0:1], xbk,
                             start=(k == 0), stop=(k == KT - 1))
            nc.tensor.matmul(s2, ones[:, 0:1], xsqk,
                             start=(k == 0), stop=(k == KT - 1))
        mu_f = lnsmall.tile([1, RB], BF16, tag="mu_f")
        mu_f2 = lnsmall.tile([1, RB], F32, tag="lntmp")
        var_f = lnsmall.tile([1, RB], F32, tag="var_f")
        rstd_bf = lnsmall.tile([1, RB], BF16, tag="rstd_bf")
        nc.vector.tensor_scalar_mul(mu_f, s1, 1.0 / C)
        nc.vector.tensor_scalar_mul(var_f, s2, 1.0 / C)
        nc.vector.tensor_mul(mu_f2, mu_f, mu_f)
        nc.vector.scalar_tensor_tensor(out=var_f, in0=var_f, scalar=EPS,
                                       in1=mu_f2, op0=ALU.add,
                                       op1=ALU.subtract)
        nc.scalar.activation(out=var_f, in_=var_f, func=AF.Sqrt, bias=0.0)
        nc.vector.reciprocal(rstd_bf, var_f)
        mu_ps = pbc.tile([P, RB], F32, tag="bc")
        rs_ps = pbc.tile([P, RB], F32, tag="bc")
        nc.tensor.matmul(mu_ps, ones[0:1, :], mu_f, start=True, stop=True)
        nc.tensor.matmul(rs_ps, ones[0:1, :], rstd_bf, start=True, stop=True)
        mu_sb = lnsmall.tile([P, RB], BF16, tag="mu_sb")
        rs_sb = lnsmall.tile([P, RB], BF16, tag="rs_sb")
        nc.vector.tensor_copy(out=mu_sb, in_=mu_ps)
        nc.vector.tensor_copy(out=rs_sb, in_=rs_ps)
        tmp = lnsmall.tile([P, RB], F32, tag="lntmp")
        for k in range(KT):
            nc.vector.tensor_sub(tmp, src[:, k, :], mu_sb)
            nc.vector.tensor_mul(tmp, tmp, rs_sb)
            nc.vector.tensor_scalar(out=xn_out[:, k, :], in0=tmp,
                                    scalar1=g[:, k:k + 1], scalar2=b[:, k:k + 1],
                                    op0=ALU.mult, op1=ALU.add)

    def proj_pass(ppk, wt_sb, bias, rhs_src, out_cb, cs):
        """out_cb[:, m, cs] = W.T @ rhs + bias for all 8 m-tiles (bf16 out)."""
        for m in range(KT):
            pk = ppk.tile([P, RB], F32, tag="pk")
            for k in range(KT):
                nc.tensor.matmul(pk, wt_sb[:, k, m * P:(m + 1) * P],
                                 rhs_src[:, k, :],
                                 start=(k == 0), stop=(k == KT - 1))
            nc.vector.tensor_scalar_add(out_cb[:, m, cs], pk, bias[:, m:m + 1])

    def build_v(ppv, vbuf, st, lhs_src, ss, w_sb):
        pv = ppv.tile([P, C], F32, tag="pv")
        for k in range(KT):
            for n2 in range(2):
                nc.tensor.matmul(pv[:, n2 * 512:(n2 + 1) * 512],
                                 lhs_src[:, k, ss * P:(ss + 1) * P],
                                 w_sb[:, k, n2 * 512:(n2 + 1) * 512],
                                 start=(k == 0), stop=(k == KT - 1))
        nc.vector.tensor_copy(out=vbuf[:, st, :, 0:HS],
                              in_=pv.rearrange("p (h d) -> p h d", h=H))
        nc.vector.memset(vbuf[:, st, :, HS:HS + 1], 1.0)

    def load_w(pool, name, tag="w"):
        t = pool.tile([P, KT, C], BF16, tag=tag)
        src = d[name].rearrange("(k p) m -> p k m", p=P)
        for k in range(KT):
            nc.sync.dma_start(out=t[:, k, :], in_=src[:, k, :])
        return t

    def dma_in_3d(dst, src):
        for k in range(dst.shape[1]):
            nc.sync.dma_start(out=dst[:, k, :], in_=src[:, k, :])

    def attention(qb, vb, kb, bv_cv, masked, htb):
        with tc.tile_pool(name="psc", bufs=2, space="PSUM") as psc, \
             tc.tile_pool(name="po", bufs=4, space="PSUM") as po:
            for hp in range(KT):
                sc = psc.tile([P, 2, RB], F32, tag="sc")
                o0 = po.tile([HS + 1, RB], F32, tag="o")
                o1 = po.tile([HS + 1, RB], F32, tag="o")
                otiles = (o0, o1)
                for st in range(ST):
                    for i in (0, 1):
                        nc.tensor.matmul(
                            sc[:, i, :],
                            kb[64 * i:64 * (i + 1), hp, st * P:(st + 1) * P],
                            qb[64 * i:64 * (i + 1), hp, :],
                            start=True, stop=True)
                    eb = epool.tile([P, 2, RB], BF16, tag="e")
                    bias_ap = sb_sbias[:, st:st + 1] if masked else sb_sbias[:, 15:16]
                    nc.scalar.activation(out=eb, in_=sc, func=AF.Exp,
                                         bias=bias_ap, scale=0.125)
                    if masked and st >= 12:
                        for i in (0, 1):
                            nc.vector.tensor_mul(eb[:, i, :], eb[:, i, :],
                                                 sb_smask[:, st - 12, :])
                    for i in (0, 1):
                        nc.tensor.matmul(otiles[i], vb[:, st, 2 * hp + i, :],
                                         eb[:, i, :],
                                         start=(st == 0), stop=(st == ST - 1))
                for i in (0, 1):
                    rzt = epool.tile([P, RB], BF16, tag="rzt")
                    nc.vector.reciprocal(rzt[HS:HS + 1, :],
                                         otiles[i][HS:HS + 1, :])
                    bc = po.tile([64, RB], F32, tag="o")
                    nc.tensor.matmul(bc, ones[HS:HS + 1, 0:64],
                                     rzt[HS:HS + 1, :], start=True, stop=True)
                    bcs = epool.tile([64, RB], BF16, tag="bcs")
                    nc.vector.tensor_copy(out=bcs, in_=bc)
                    if i == 0:
                        nc.vector.tensor_mul(htb[0:64, hp, :],
                                             otiles[i][0:HS, :], bcs)
                        nc.vector.tensor_scalar_add(
                            htb[0:64, hp, :], htb[0:64, hp, :],
                            bv_cv[0:64, hp:hp + 1])
                    else:
                        htmp = epool.tile([64, RB], BF16, tag="htmp")
                        nc.vector.tensor_mul(htmp, otiles[i][0:HS, :], bcs)
                        nc.vector.tensor_scalar_add(htmp, htmp,
                                                    bv_cv[64:128, hp:hp + 1])
                        nc.sync.dma_start(out=htb[64:128, hp, :], in_=htmp)

    def proj_residual(wname, bias_cv, htb, res_src_fn, res_out):
        """res_out[:,m,:] = W.T @ h + b + res_src_fn(m)."""
        with tc.tile_pool(name="pw_pr", bufs=1) as pw, \
             tc.tile_pool(name="ppr", bufs=2, space="PSUM") as ppm:
            w_sb = load_w(pw, wname, tag="wpr")
            for m in range(KT):
                pp = ppm.tile([P, RB], F32, tag="pp")
                for k in range(KT):
                    nc.tensor.matmul(pp, w_sb[:, k, m * P:(m + 1) * P],
                                     htb[:, k, :],
                                     start=(k == 0), stop=(k == KT - 1))
                nc.vector.tensor_scalar_add(res_out[:, m, :], pp,
                                            bias_cv[:, m:m + 1])
                nc.vector.tensor_add(res_out[:, m, :], res_out[:, m, :],
                                     res_src_fn(m))

    xTr = d["xT"].rearrange("(k p) s -> p k s", p=P)
    res1 = None
    res2 = None
    with tc.tile_pool(name="kpool", bufs=1) as kpool, \
         tc.tile_pool(name="vpool", bufs=1) as vpool, \
         tc.tile_pool(name="qpool", bufs=1) as qpool, \
         tc.tile_pool(name="hpool", bufs=1) as hpool, \
         tc.tile_pool(name="xocp", bufs=2) as xocp:
        kbuf = kpool.tile([P, KT, T], BF16, tag="k")
        vbuf = vpool.tile([P, ST, H, HS + 1], BF16, tag="v")
        qbuf = qpool.tile([P, KT, RB], BF16, tag="q")
        htb = hpool.tile([P, KT, RB], BF16, tag="h")

        # ======== Phase A: ln1 + self K/Q/V, single pass over 4 chunks ======
        with tc.tile_pool(name="pa_stat", bufs=2, space="PSUM") as pstat, \
             tc.tile_pool(name="pa_bc", bufs=2, space="PSUM") as pbc, \
             tc.tile_pool(name="pa_k", bufs=2, space="PSUM") as ppk, \
             tc.tile_pool(name="pa_v", bufs=1, space="PSUM") as ppv, \
             tc.tile_pool(name="pa_sb", bufs=1) as sbtmp, \
             tc.tile_pool(name="pa_w", bufs=2) as watmp:
            wsk_sb = load_w(watmp, "wsk")
            wsv_sb = load_w(watmp, "wsv")
            xn = None
            for c4 in range(4):
                cs = slice(c4 * RB, (c4 + 1) * RB)
                xb = sbtmp.tile([P, KT, RB], BF16, tag="xb")
                dma_in_3d(xb, xTr[:, :, cs])
                xn = sbtmp.tile([P, KT, RB], BF16, tag="xn")
                ln_apply((pstat, pbc, sbtmp), xb, True, xn, cv["g1"], cv["t1"])
                proj_pass(ppk, wsk_sb, cv["bsk"], xn, kbuf, cs)
                for ss in range(4):
                    build_v(ppv, vbuf, 4 * c4 + ss, xn, ss, wsv_sb)
            # Q from chunk 3's xn (own rows), after wsk slot is free
            wsq_sb = load_w(watmp, "wsq")
            proj_pass(ppk, wsq_sb, cv["bsq"], xn, qbuf, slice(0, RB))

        # ==================== self-attention + m_proj =======================
        attention(qbuf, vbuf, kbuf, cv["bsv"], True, htb)
        res1 = respool.tile([P, KT, RB], F32, tag="res")

        def self_res(m):
            xoc = xocp.tile([P, RB], BF16, tag="xoc")
            nc.sync.dma_start(out=xoc, in_=xTr[:, m, 3 * RB:T])
            return xoc

        proj_residual("wmp", cv["bmp"], htb, self_res, res1)

        # ===================== Phase C: cross-attention =====================
        with tc.tile_pool(name="pc_stat", bufs=2, space="PSUM") as pstat, \
             tc.tile_pool(name="pc_bc", bufs=2, space="PSUM") as pbc, \
             tc.tile_pool(name="pc_k", bufs=2, space="PSUM") as ppk, \
             tc.tile_pool(name="pc_v", bufs=1, space="PSUM") as ppv, \
             tc.tile_pool(name="pc_sb", bufs=2) as sbtmp, \
             tc.tile_pool(name="pc_w", bufs=1) as watmp:
            xn2 = xnpool.tile([P, KT, RB], BF16, tag="xn2")
            ln_apply((pstat, pbc, sbtmp), res1, False, xn2, cv["g2"], cv["t2"])
            wcq_sb = load_w(watmp, "wcq")
            proj_pass(ppk, wcq_sb, cv["bcq"], xn2, qbuf, slice(0, RB))
            xer = d["xeT"].rearrange("(k p) s -> p k s", p=P)
            wck_sb = load_w(watmp, "wck")
            for c4 in range(4):
                cs = slice(c4 * RB, (c4 + 1) * RB)
                xec = sbtmp.tile([P, KT, RB], BF16, tag="xec")
                dma_in_3d(xec, xer[:, :, cs])
                proj_pass(ppk, wck_sb, cv["bck"], xec, kbuf, cs)
            wcv_sb = load_w(watmp, "wcv")
            for c4 in range(4):
                cs = slice(c4 * RB, (c4 + 1) * RB)
                xec = sbtmp.tile([P, KT, RB], BF16, tag="xec")
                dma_in_3d(xec, xer[:, :, cs])
                for ss in range(4):
                    build_v(ppv, vbuf, 4 * c4 + ss, xec, ss, wcv_sb)

        attention(qbuf, vbuf, kbuf, cv["bcv"], False, htb)
        res2 = respool.tile([P, KT, RB], F32, tag="res")
        proj_residual("wcp", cv["bcp"], htb, lambda m: res1[:, m, :], res2)

    # ================================ FFN =================================
    with tc.tile_pool(name="pf_stat", bufs=2, space="PSUM") as pstat, \
         tc.tile_pool(name="pf_bc", bufs=2, space="PSUM") as pbc, \
         tc.tile_pool(name="pf_h", bufs=2, space="PSUM") as pph, \
         tc.tile_pool(name="pf_sb", bufs=2) as sbtmp, \
         tc.tile_pool(name="pf_w", bufs=2) as watmp, \
         tc.tile_pool(name="pf_h1", bufs=1) as h1pool:
        xn3 = xnpool.tile([P, KT, RB], BF16, tag="xn2")
        ln_apply((pstat, pbc, sbtmp), res2, False, xn3, cv["g3"], cv["t3"])
        h1 = h1pool.tile([P, 32, RB], BF16, tag="h1")
        wf1r = d["wf1"].rearrange("(k p) m -> p k m", p=P)
        for mg in range(4):
            wg = watmp.tile([P, KT, C], BF16, tag="w")
            dma_in_3d(wg, wf1r[:, :, mg * C:(mg + 1) * C])
            for mm in range(KT):
                m = mg * KT + mm
                pp = pph.tile([P, RB], F32, tag="pp")
                for k in range(KT):
                    nc.tensor.matmul(pp, wg[:, k, mm * P:(mm + 1) * P],
                                     xn3[:, k, :],
                                     start=(k == 0), stop=(k == KT - 1))
                nc.vector.tensor_scalar(out=h1[:, m, :], in0=pp,
                                        scalar1=cv["bf1"][:, m:m + 1],
                                        scalar2=0.0,
                                        op0=ALU.add, op1=ALU.max)
        wf2r = d["wf2"].rearrange("(k p) m -> p k m", p=P)
        outr = d["outT"].rearrange("(k p) q -> p k q", p=P)
        oT = respool.tile([P, KT, RB], F32, tag="res")
        for m in range(KT):
            wg2 = watmp.tile([P, 32, P], BF16, tag="w")
            src2 = wf2r[:, :, m * P:(m + 1) * P]
            for k4 in range(4):
                nc.sync.dma_start(out=wg2[:, 8 * k4:8 * (k4 + 1), :],
                                  in_=src2[:, 8 * k4:8 * (k4 + 1), :])
            pp = pph.tile([P, RB], F32, tag="pp")
            for k in range(32):
                nc.tensor.matmul(pp, wg2[:, k, :], h1[:, k, :],
                                 start=(k == 0), stop=(k == 31))
            nc.vector.tensor_scalar_add(oT[:, m, :], pp, cv["bf2"][:, m:m + 1])
            nc.vector.tensor_add(oT[:, m, :], oT[:, m, :], res2[:, m, :])
            nc.sync.dma_start(out=outr[:, m, :], in_=oT[:, m, :])

    ctx.close()


_NC_CACHE = None


def _get_nc():
    global _NC_CACHE
    if _NC_CACHE is None:
        _NC_CACHE = _build_nc()
    return _NC_CACHE


def _heads_concat(w):
    return np.ascontiguousarray(np.transpose(np.asarray(w), (1, 0, 2))
                                .reshape(C, C))


def kernel(**inputs):
    inp = {k: np.asarray(v) for k, v in inputs.items()}
    nc = _get_nc()

    shared = {
        "wsq": _heads_concat(inp["mq_w"]).astype(BF),
        "wsk": _heads_concat(inp["mk_w"]).astype(BF),
        "wsv": _heads_concat(inp["mv_w"]).astype(BF),
        "wcq": _heads_concat(inp["cq_w"]).astype(BF),
        "wck": _heads_concat(inp["ck_w"]).astype(BF),
        "wcv": _heads_concat(inp["cv_w"]).astype(BF),
        "wmp": inp["m_proj_w"].astype(BF),
        "wcp": inp["c_proj_w"].astype(BF),
        "wf1": inp["f_w1"].astype(BF),
        "wf2": inp["f_w2"].astype(BF),
        "bsq": inp["mq_b"].reshape(C).astype(np.float32),
        "bsk": inp["mk_b"].reshape(C).astype(np.float32),
        "bsv": inp["mv_b"].reshape(C).astype(np.float32),
        "bcq": inp["cq_b"].reshape(C).astype(np.float32),
        "bck": inp["ck_b"].reshape(C).astype(np.float32),
        "bcv": inp["cv_b"].reshape(C).astype(np.float32),
        "bmp": inp["m_proj_b"].astype(np.float32),
        "bcp": inp["c_proj_b"].astype(np.float32),
        "bf1": inp["f_b1"].astype(np.float32),
        "bf2": inp["f_b2"].astype(np.float32),
        "g1": inp["ln1_g"].astype(np.float32),
        "t1": inp["ln1_b"].astype(np.float32),
        "g2": inp["ln2_g"].astype(np.float32),
        "t2": inp["ln2_b"].astype(np.float32),
        "g3": inp["ln3_g"].astype(np.float32),
        "t3": inp["ln3_b"].astype(np.float32),
        "smask": np.triu(np.ones((RB, RB), np.float32)).astype(BF),
    }

    x = inp["x"].astype(np.float32)
    xe = inp["x_e"].astype(np.float32)
    in_maps = []
    for core in range(NCORE):
        b, j = core // 4, core % 4
        q0 = j * RB
        perm = np.concatenate([np.arange(0, q0),
                               np.arange(q0 + RB, T),
                               np.arange(q0, q0 + RB)])
        sb = np.zeros(T, np.float32)
        sb[q0:T - RB] = NEG
        m = dict(shared)
        m["xT"] = np.ascontiguousarray(x[b][perm].T).astype(BF)
        m["xeT"] = np.ascontiguousarray(xe[b].T).astype(BF)
        m["sbias"] = sb
        in_maps.append(m)

    res = run_bass_kernel_spmd(nc, in_maps, core_ids=list(range(NCORE)))
    out = np.empty((B, T, C), np.float32)
    for core in range(NCORE):
        b, j = core // 4, core % 4
        out[b, j * RB:(j + 1) * RB, :] = res.results[core]["outT"].T
    return out



# revision 10
# speedup vs baseline: 1.8114x; 1.8114x over previous
"""Trainium2 Bass kernel for a dense transformer decoder block.

Sharding: sequence-parallel over B*T rows -> 8 cores (batch = core//4,
j = core%4). Core j owns the strided token subset {j+4i} of its batch;
tokens are host-permuted within each 512-chunk (own 128 tokens first),
which makes the causal structure identical across cores: q-tile qi needs
s-tiles 0..4qi+3 only, with a per-core host-provided mask on the last 4
(diagonal) s-tiles. This enables true causal skipping in a uniform SPMD
program while keeping per-core causal work balanced.

Precision: the whole attention path runs fp8e4 matmul operands with the
DoubleRow perf mode (2 k-tiles / 256-contraction per instruction);
weights are host-prescaled by 64 (fp8e4 min-normal is 2^-6; raw weights
have std 0.02). Scores/PSUM/residual/LN stats stay fp32; probabilities
are quantized UNNORMALIZED (exp values are O(1); normalized probs would
be fp8 subnormals). FFN stays bf16 (fp8 there costs ~2e-2 rel err).
LN gamma/beta and all attention V/proj biases are folded host-side into
weights / downstream bias columns; Q/K biases ride along in the existing
PSUM->fp8 cast ops. Broadcasts run on the idle GpSimd(Pool) engine.
Validated ~3.3e-3 rel err vs the fp32 reference in numpy simulation.
"""

import sys
import numpy as np

sys.path.insert(0, "/opt/trn_rl_repo")

import ml_dtypes  # noqa: E402
import concourse.bass as bass  # noqa: E402
import concourse.bacc as bacc  # noqa: E402
import concourse.tile as tile  # noqa: E402
from concourse import mybir  # noqa: E402
from concourse.bass_utils import run_bass_kernel_spmd  # noqa: E402

BFH = ml_dtypes.bfloat16
F8H = ml_dtypes.float8_e4m3     # TRN float8e4 (max +-240)
F32 = mybir.dt.float32
BF16 = mybir.dt.bfloat16
F8 = mybir.dt.float8e4
AF = mybir.ActivationFunctionType
ALU = mybir.AluOpType
DRM = mybir.MatmulPerfMode.DoubleRow

B, T, SE, C, H, HS = 2, 2048, 2048, 1024, 16, 64
NCORE = 8
RB = 512            # own tokens per core
KT = 8              # 128-slabs over C
KP = 4              # DoubleRow k-pairs over C
ST = 16             # 128-tiles over T / SE
P = 128
EPS = 1e-5
WS = 64.0                       # fp8 weight prescale
SC_QK = 0.125 / (WS * WS)       # exp input scale (undo 64*64)
IPROJ = 1.0 / (WS * WS)         # proj psum descale


def _build_nc():
    nc = bacc.Bacc(None, target_bir_lowering=False)

    def din(name, shape, dt):
        return nc.dram_tensor(name, shape, dt, kind="ExternalInput").ap()

    d = {}
    d["xT"] = din("xT", [C, T], BF16)          # per-core permuted x^T
    d["xoT"] = din("xoT", [C, RB], BF16)       # own rows (+ m_proj bias)
    d["xeT"] = din("xeT", [C, SE], F8)         # x_e^T fp8
    d["dmask"] = din("dmask", [P, 4, 2, P], F8)
    for n in ["wsq", "wsk", "wsv", "wcq", "wck", "wcv", "wmp", "wcp"]:
        d[n] = din(n, [C, C], F8)
    d["wf1"] = din("wf1", [C, 4 * C], BF16)
    d["wf2"] = din("wf2", [4 * C, C], BF16)
    for n in ["bsq", "bsk", "bcq", "bck", "bcp", "bf2"]:
        d[n] = din(n, [C], F32)
    d["bf1"] = din("bf1", [4 * C], F32)
    d["outT"] = nc.dram_tensor("outT", [C, RB], F32, kind="ExternalOutput").ap()

    with tile.TileContext(nc) as tc:
        _emit(tc, nc, d)
    nc.finalize()
    return nc


def _emit(tc, nc, d):
    from contextlib import ExitStack
    ctx = ExitStack()
    ctx.enter_context(nc.allow_low_precision(reason="fp8/bf16 matmul operands"))

    # ---------------- persistent pools ----------------
    consts = ctx.enter_context(tc.tile_pool(name="consts", bufs=1))
    respool = ctx.enter_context(tc.tile_pool(name="respool", bufs=1))
    xnpool = ctx.enter_context(tc.tile_pool(name="xnpool", bufs=1))
    epool = ctx.enter_context(tc.tile_pool(name="epool", bufs=2))
    small = ctx.enter_context(tc.tile_pool(name="small", bufs=2))
    qpool = ctx.enter_context(tc.tile_pool(name="qpool", bufs=2))
    hpool = ctx.enter_context(tc.tile_pool(name="hpool", bufs=1))
    wffn = ctx.enter_context(tc.tile_pool(name="wffn", bufs=2))
    ones = consts.tile([P, P], BF16)
    nc.vector.memset(ones, 1.0)
    dmsk = consts.tile([P, 4, 2, P], F8)
    nc.sync.dma_start(out=dmsk, in_=d["dmask"])

    def colvec(name1d, n=KT):
        t = consts.tile([P, n], F32, tag=f"cv_{name1d}")
        nc.sync.dma_start(out=t, in_=d[name1d].rearrange("(m p) -> p m", p=P))
        return t

    cv = {n: colvec(n) for n in ["bsq", "bsk", "bcq", "bck", "bcp", "bf2"]}
    cv["bf1"] = colvec("bf1", 32)

    def load_w(pool, name, tag="w", dt=F8, width=C):
        t = pool.tile([P, KT, width], dt, tag=tag)
        src = d[name].rearrange("(k p) m -> p k m", p=P)
        for k in range(KT):
            nc.sync.dma_start(out=t[:, k, :], in_=src[:, k, :])
        return t

    def ln_stats(pstat, src_bf):
        """Mean/rstd over C for RB tokens from bf16 src [P, KT, RB].
        Returns (mu_sb, rs_sb) bf16 [P, RB] broadcast tiles (Pool bcast)."""
        s1 = pstat.tile([1, RB], F32, tag="s1")
        s2 = pstat.tile([1, RB], F32, tag="s1")
        for k in range(KT):
            xsqk = epool.tile([P, RB], BF16, tag="xsqk")
            nc.vector.tensor_mul(xsqk, src_bf[:, k, :], src_bf[:, k, :])
            nc.tensor.matmul(s1, ones[:, 0:1], src_bf[:, k, :],
                             start=(k == 0), stop=(k == KT - 1))
            nc.tensor.matmul(s2, ones[:, 0:1], xsqk,
                             start=(k == 0), stop=(k == KT - 1))
        mu_f = small.tile([1, RB], BF16, tag="mu_f")
        mu_f2 = small.tile([1, RB], F32, tag="lntmp")
        var_f = small.tile([1, RB], F32, tag="lntmp")
        rstd_bf = small.tile([1, RB], BF16, tag="rstd_bf")
        nc.vector.tensor_scalar_mul(mu_f, s1, 1.0 / C)
        nc.vector.tensor_scalar_mul(var_f, s2, 1.0 / C)
        nc.vector.tensor_mul(mu_f2, mu_f, mu_f)
        nc.vector.scalar_tensor_tensor(out=var_f, in0=var_f, scalar=EPS,
                                       in1=mu_f2, op0=ALU.add,
                                       op1=ALU.subtract)
        nc.scalar.activation(out=var_f, in_=var_f, func=AF.Sqrt, bias=0.0)
        nc.vector.reciprocal(rstd_bf, var_f)
        mu_sb = small.tile([P, RB], BF16, tag="mu_sb")
        rs_sb = small.tile([P, RB], BF16, tag="rs_sb")
        nc.gpsimd.partition_broadcast(mu_sb, mu_f)
        nc.gpsimd.partition_broadcast(rs_sb, rstd_bf)
        return mu_sb, rs_sb

    def ln_apply(src_bf, mu_sb, rs_sb, xn_out):
        """xn = (src - mu) * rstd; sub on DVE, mul (+cast) on Pool."""
        for k in range(KT):
            tmp = epool.tile([P, RB], BF16, tag="lnt")
            nc.vector.tensor_sub(tmp, src_bf[:, k, :], mu_sb)
            nc.gpsimd.tensor_mul(xn_out[:, k, :], tmp, rs_sb)

    def proj_dr(ppk, w_sb, bias_cv, rhs, out_cb, cs, tag="pk"):
        """out_cb[:, m, cs] = fp8(W.T @ rhs + bias) via DoubleRow."""
        for m in range(KT):
            pk = ppk.tile([P, RB], F32, tag=tag)
            for kp in range(KP):
                nc.tensor.matmul(pk, w_sb[:, 2 * kp:2 * kp + 2,
                                          m * P:(m + 1) * P],
                                 rhs[:, 2 * kp:2 * kp + 2, :],
                                 start=(kp == 0), stop=(kp == KP - 1),
                                 perf_mode=DRM)
            nc.vector.tensor_scalar_add(out_cb[:, m, cs], pk,
                                        bias_cv[:, m:m + 1])

    def build_v(ppv, vb, st, xn, ss, w_sb):
        """vb[:, st, :, 0:64] = fp8 of (xn_tokens @ W) for s-tile st."""
        pv = ppv.tile([P, C], F32, tag="pv")
        for kp in range(KP):
            lhs = xn[:, 2 * kp:2 * kp + 2, ss * P:(ss + 1) * P]
            for n2 in range(2):
                nc.tensor.matmul(pv[:, n2 * 512:(n2 + 1) * 512],
                                 lhs, w_sb[:, 2 * kp:2 * kp + 2,
                                           n2 * 512:(n2 + 1) * 512],
                                 start=(kp == 0), stop=(kp == KP - 1),
                                 perf_mode=DRM)
        nc.scalar.activation(out=vb[:, st, :, 0:HS],
                             in_=pv.rearrange("p (h d) -> p h d", h=H),
                             func=AF.Copy)

    def norm_write(o, hp, i, qw, qn, htb_dst):
        """htb_dst[64i:64i+64, hp, qw] = fp8(o[0:64]/o[64])."""
        rz = small.tile([1, RB], BF16, tag="rz")
        nc.vector.reciprocal(rz[:, 0:qn], o[HS:HS + 1, i, 0:qn])
        bcs = small.tile([HS, RB], BF16, tag="bcs")
        nc.gpsimd.partition_broadcast(bcs[:, 0:qn], rz[:, 0:qn])
        nc.vector.tensor_mul(htb_dst[HS * i:HS * (i + 1), hp, qw],
                             o[0:HS, i, 0:qn], bcs[:, 0:qn])

    xTr = d["xT"].rearrange("(k p) s -> p k s", p=P)
    xer = d["xeT"].rearrange("(k p) s -> p k s", p=P)
    wf1r = d["wf1"].rearrange("(k p) m -> p k m", p=P)

    qbuf = qpool.tile([P, KT, RB], F8, tag="q")
    htb = hpool.tile([P, KT, RB], F8, tag="h")
    res1 = respool.tile([P, KT, RB], BF16, tag="res1")
    res2 = respool.tile([P, KT, RB], BF16, tag="res2")

    with tc.tile_pool(name="kpool", bufs=1) as kpool, \
         tc.tile_pool(name="vpool", bufs=1) as vpool:
        kbuf = kpool.tile([P, KT, T], F8, tag="k")       # self K
        vbuf = vpool.tile([P, ST, H, HS + 1], F8, tag="v")
        nc.vector.memset(vbuf[:, :, :, HS:HS + 1], 1.0)

        # ============ Phase A: ln1 + self K/V/Q over 4 chunks ============
        with tc.tile_pool(name="pa_stat", bufs=2, space="PSUM") as pstat, \
             tc.tile_pool(name="pa_k", bufs=2, space="PSUM") as ppk, \
             tc.tile_pool(name="pa_v", bufs=1, space="PSUM") as ppv, \
             tc.tile_pool(name="pa_sb", bufs=2) as sbtmp, \
             tc.tile_pool(name="pa_w", bufs=3) as watmp:
            wsk_sb = load_w(watmp, "wsk")
            wsv_sb = load_w(watmp, "wsv")
            wsq_sb = load_w(watmp, "wsq")
            for c4 in range(4):
                cs = slice(c4 * RB, (c4 + 1) * RB)
                xb = sbtmp.tile([P, KT, RB], BF16, tag="xb")
                for k in range(KT):
                    nc.sync.dma_start(out=xb[:, k, :], in_=xTr[:, k, cs])
                mu_sb, rs_sb = ln_stats(pstat, xb)
                xn = sbtmp.tile([P, KT, RB], F8, tag="xn")
                ln_apply(xb, mu_sb, rs_sb, xn)
                proj_dr(ppk, wsk_sb, cv["bsk"], xn, kbuf, cs)
                for ss in range(4):
                    build_v(ppv, vbuf, 4 * c4 + ss, xn, ss, wsv_sb)
                # Q for this chunk's own 128 tokens (permuted to cols 0:128)
                for m in range(KT):
                    pq = ppk.tile([P, RB], F32, tag="pk")
                    for kp in range(KP):
                        nc.tensor.matmul(pq[:, 0:P],
                                         wsq_sb[:, 2 * kp:2 * kp + 2,
                                                m * P:(m + 1) * P],
                                         xn[:, 2 * kp:2 * kp + 2, 0:P],
                                         start=(kp == 0), stop=(kp == KP - 1),
                                         perf_mode=DRM)
                    nc.vector.tensor_scalar_add(
                        qbuf[:, m, c4 * P:(c4 + 1) * P], pq[:, 0:P],
                        cv["bsq"][:, m:m + 1])

        # prefetch first FFN W1 group (DMA overlaps attention phases)
        wg1_0 = wffn.tile([P, KT, C], BF16, tag="wg1")
        for k in range(KT):
            nc.sync.dma_start(out=wg1_0[:, k, :], in_=wf1r[:, k, 0:C])

        # ===== self-attention (causal-skipped) + interleaved cross K/V =====
        with tc.tile_pool(name="pb_sc", bufs=2, space="PSUM") as psc, \
             tc.tile_pool(name="pb_o", bufs=1, space="PSUM") as po, \
             tc.tile_pool(name="pb_sb", bufs=2) as sbtmp, \
             tc.tile_pool(name="pb_w", bufs=2) as watmp:
            kbuf2 = kpool.tile([P, KT, T], F8, tag="k2")     # cross K
            vbuf2 = vpool.tile([P, ST, H, HS + 1], F8, tag="v2")
            nc.vector.memset(vbuf2[:, :, :, HS:HS + 1], 1.0)
            wck_sb = load_w(watmp, "wck")
            wcv_sb = load_w(watmp, "wcv")

            def cross_kv_chunk(c4):
                cs = slice(c4 * RB, (c4 + 1) * RB)
                xec = sbtmp.tile([P, KT, RB], F8, tag="xec")
                for k in range(KT):
                    nc.sync.dma_start(out=xec[:, k, :], in_=xer[:, k, cs])
                proj_dr(psc, wck_sb, cv["bck"], xec, kbuf2, cs)
                for ss in range(4):
                    build_v(po, vbuf2, 4 * c4 + ss, xec, ss, wcv_sb)

            for hp in range(KT):
                for qi in range(4):
                    nst = 4 * qi + 4
                    eb = sbtmp.tile([P, nst, 2, P], F8, tag=f"eb{qi}")
                    for g in range(nst // 2):
                        sc = psc.tile([P, 2, 2, P], F32, tag="sc")
                        for k2 in range(2):
                            st = 2 * g + k2
                            for i in (0, 1):
                                nc.tensor.matmul(
                                    sc[:, k2, i, :],
                                    kbuf[HS * i:HS * (i + 1), hp,
                                         st * P:(st + 1) * P],
                                    qbuf[HS * i:HS * (i + 1), hp,
                                         qi * P:(qi + 1) * P],
                                    start=True, stop=True)
                        nc.scalar.activation(out=eb[:, 2 * g:2 * g + 2, :, :],
                                             in_=sc, func=AF.Exp, scale=SC_QK)
                    # diagonal mask on the last 4 s-tiles (one Pool op)
                    nc.gpsimd.tensor_mul(eb[:, nst - 4:nst, :, :],
                                         eb[:, nst - 4:nst, :, :], dmsk)
                    o = po.tile([HS + 1, 2, RB], F32, tag="o")
                    for i in (0, 1):
                        for sp in range(nst // 2):
                            nc.tensor.matmul(
                                o[:, i, 0:P],
                                vbuf[:, 2 * sp:2 * sp + 2, 2 * hp + i,
                                     0:HS + 1],
                                eb[:, 2 * sp:2 * sp + 2, i, :],
                                start=(sp == 0), stop=(sp == nst // 2 - 1),
                                perf_mode=DRM)
                    for i in (0, 1):
                        norm_write(o, hp, i, slice(qi * P, (qi + 1) * P), P,
                                   htb)
                if hp < 4:
                    cross_kv_chunk(hp)

        # =================== m_proj + residual 1 ===================
        with tc.tile_pool(name="pm_k", bufs=2, space="PSUM") as ppk, \
             tc.tile_pool(name="pm_w", bufs=1) as watmp, \
             tc.tile_pool(name="pm_sb", bufs=1) as sbtmp:
            wmp_sb = load_w(watmp, "wmp")
            xoc = sbtmp.tile([P, KT, RB], BF16, tag="xoc")
            xor_ = d["xoT"].rearrange("(k p) s -> p k s", p=P)
            for k in range(KT):
                nc.sync.dma_start(out=xoc[:, k, :], in_=xor_[:, k, :])
            for m in range(KT):
                pp = ppk.tile([P, RB], F32, tag="pk")
                for kp in range(KP):
                    nc.tensor.matmul(pp, wmp_sb[:, 2 * kp:2 * kp + 2,
                                                m * P:(m + 1) * P],
                                     htb[:, 2 * kp:2 * kp + 2, :],
                                     start=(kp == 0), stop=(kp == KP - 1),
                                     perf_mode=DRM)
                nc.vector.scalar_tensor_tensor(out=res1[:, m, :], in0=pp,
                                               scalar=IPROJ,
                                               in1=xoc[:, m, :],
                                               op0=ALU.mult, op1=ALU.add)

        # ================= ln2 + cross Q ==================
        xn2 = xnpool.tile([P, KT, RB], F8, tag="xn2")
        qbuf2 = qpool.tile([P, KT, RB], F8, tag="q")
        with tc.tile_pool(name="pc_stat", bufs=2, space="PSUM") as pstat, \
             tc.tile_pool(name="pc_k", bufs=2, space="PSUM") as ppk, \
             tc.tile_pool(name="pc_w", bufs=1) as watmp, \
             tc.tile_pool(name="pc_sb", bufs=1) as sbtmp:
            mu_sb, rs_sb = ln_stats(pstat, res1)
            ln_apply(res1, mu_sb, rs_sb, xn2)
            wcq_sb = load_w(watmp, "wcq")
            proj_dr(ppk, wcq_sb, cv["bcq"], xn2, qbuf2, slice(0, RB))

        # ======================== cross-attention ========================
        with tc.tile_pool(name="px_sc", bufs=2, space="PSUM") as psc, \
             tc.tile_pool(name="px_o", bufs=1, space="PSUM") as po, \
             tc.tile_pool(name="px_sb", bufs=2) as sbtmp:
            for hp in range(KT):
                o = po.tile([HS + 1, 2, RB], F32, tag="ox")
                for sp in range(8):
                    eb = sbtmp.tile([P, 2, 2, RB], F8, tag="ebx")
                    for k2 in (0, 1):
                        st = 2 * sp + k2
                        sc = psc.tile([P, 2, RB], F32, tag="scx")
                        for i in (0, 1):
                            nc.tensor.matmul(
                                sc[:, i, :],
                                kbuf2[HS * i:HS * (i + 1), hp,
                                      st * P:(st + 1) * P],
                                qbuf2[HS * i:HS * (i + 1), hp, :],
                                start=True, stop=True)
                        nc.scalar.activation(out=eb[:, k2, :, :], in_=sc,
                                             func=AF.Exp, scale=SC_QK)
                    for i in (0, 1):
                        nc.tensor.matmul(
                            o[:, i, :],
                            vbuf2[:, 2 * sp:2 * sp + 2, 2 * hp + i, 0:HS + 1],
                            eb[:, :, i, :],
                            start=(sp == 0), stop=(sp == 7),
                            perf_mode=DRM)
                for i in (0, 1):
                    norm_write(o, hp, i, slice(0, RB), RB, htb)

            # =================== c_proj + residual 2 ===================
            with tc.tile_pool(name="py_w", bufs=1) as watmp:
                wcp_sb = load_w(watmp, "wcp")
                for m in range(KT):
                    pp = psc.tile([P, 2, RB], F32, tag="scx")
                    for kp in range(KP):
                        nc.tensor.matmul(pp[:, 0, :],
                                         wcp_sb[:, 2 * kp:2 * kp + 2,
                                                m * P:(m + 1) * P],
                                         htb[:, 2 * kp:2 * kp + 2, :],
                                         start=(kp == 0), stop=(kp == KP - 1),
                                         perf_mode=DRM)
                    tmp = small.tile([P, RB], F32, tag="cp")
                    nc.vector.scalar_tensor_tensor(out=tmp, in0=pp[:, 0, :],
                                                   scalar=IPROJ,
                                                   in1=res1[:, m, :],
                                                   op0=ALU.mult, op1=ALU.add)
                    nc.vector.tensor_scalar_add(res2[:, m, :], tmp,
                                                cv["bcp"][:, m:m + 1])

    # ================================ FFN ================================
    with tc.tile_pool(name="pf_sb", bufs=1) as sbtmp, \
         tc.tile_pool(name="pf_h1", bufs=1) as h1pool:
        h1 = h1pool.tile([P, 32, RB], BF16, tag="h1")
        with tc.tile_pool(name="pf_stat", bufs=2, space="PSUM") as pstat, \
             tc.tile_pool(name="pf_h", bufs=2, space="PSUM") as pph:
            mu_sb, rs_sb = ln_stats(pstat, res2)
            xn3 = sbtmp.tile([P, KT, RB], BF16, tag="xn3")
            ln_apply(res2, mu_sb, rs_sb, xn3)
            for mg in range(4):
                if mg == 0:
                    wg = wg1_0
                else:
                    wg = wffn.tile([P, KT, C], BF16, tag="wg1")
                    for k in range(KT):
                        nc.sync.dma_start(out=wg[:, k, :],
                                          in_=wf1r[:, k, mg * C:(mg + 1) * C])
                for mm in range(KT):
                    m = mg * KT + mm
                    pp = pph.tile([P, RB], F32, tag="pp")
                    for k in range(KT):
                        nc.tensor.matmul(pp, wg[:, k, mm * P:(mm + 1) * P],
                                         xn3[:, k, :],
                                         start=(k == 0), stop=(k == KT - 1))
                    nc.vector.tensor_scalar(out=h1[:, m, :], in0=pp,
                                            scalar1=cv["bf1"][:, m:m + 1],
                                            scalar2=0.0,
                                            op0=ALU.add, op1=ALU.max)
        wf2r = d["wf2"].rearrange("(k p) m -> p k m", p=P)
        outr = d["outT"].rearrange("(k p) q -> p k q", p=P)
        with tc.tile_pool(name="pf_acc", bufs=1, space="PSUM") as pacc:
            accs = []
            for m in range(KT):
                acc_t = pacc.tile([P, RB], F32, tag=f"acc{m}")
                accs.append(acc_t)
            for g in range(4):
                wg2 = wffn.tile([P, KT, C], BF16, tag="wg1")
                for k in range(KT):
                    nc.sync.dma_start(out=wg2[:, k, :],
                                      in_=wf2r[:, g * KT + k, :])
                for kk in range(KT):
                    for m in range(KT):
                        nc.tensor.matmul(accs[m],
                                         wg2[:, kk, m * P:(m + 1) * P],
                                         h1[:, g * KT + kk, :],
                                         start=(g == 0 and kk == 0),
                                         stop=(g == 3 and kk == KT - 1))
            for m in range(KT):
                ot = epool.tile([P, RB], F32, tag="ot")
                nc.vector.tensor_scalar_add(ot, accs[m],
                                            cv["bf2"][:, m:m + 1])
                nc.vector.tensor_add(ot, ot, res2[:, m, :])
                nc.sync.dma_start(out=outr[:, m, :], in_=ot)

    ctx.close()


_NC_CACHE = None


def _get_nc():
    global _NC_CACHE
    if _NC_CACHE is None:
        _NC_CACHE = _build_nc()
    return _NC_CACHE


def _heads_concat(w):
    return np.ascontiguousarray(np.transpose(np.asarray(w, np.float32),
                                             (1, 0, 2)).reshape(C, C))


def _f8(x):
    return np.clip(np.asarray(x, np.float32), -240.0, 240.0).astype(F8H)


def _bf(a):
    return np.ascontiguousarray(a).astype(BFH)


def kernel(**inputs):
    inp = {k: np.asarray(v, np.float32) for k, v in inputs.items()}
    nc = _get_nc()

    g1, b1 = inp["ln1_g"], inp["ln1_b"]
    g2, b2 = inp["ln2_g"], inp["ln2_b"]
    g3, b3 = inp["ln3_g"], inp["ln3_b"]
    Wq, Wk, Wv = (_heads_concat(inp["mq_w"]), _heads_concat(inp["mk_w"]),
                  _heads_concat(inp["mv_w"]))
    Cq, Ck, Cv = (_heads_concat(inp["cq_w"]), _heads_concat(inp["ck_w"]),
                  _heads_concat(inp["cv_w"]))
    bq = inp["mq_b"].reshape(C) + b1 @ Wq
    bk = inp["mk_b"].reshape(C) + b1 @ Wk
    cv_s = inp["mv_b"].reshape(C) + b1 @ Wv      # folded into m_proj bias
    bcq = inp["cq_b"].reshape(C) + b2 @ Cq
    bck = inp["ck_b"].reshape(C)
    ccv = inp["cv_b"].reshape(C)                 # folded into c_proj bias
    Wmp, Wcp = inp["m_proj_w"], inp["c_proj_w"]
    bmp = inp["m_proj_b"] + cv_s @ Wmp
    bcp = inp["c_proj_b"] + ccv @ Wcp
    W1 = g3[:, None] * inp["f_w1"]
    bf1 = inp["f_b1"] + b3 @ inp["f_w1"]

    shared = {
        "wsq": _f8(WS * g1[:, None] * Wq), "wsk": _f8(WS * g1[:, None] * Wk),
        "wsv": _f8(WS * g1[:, None] * Wv),
        "wcq": _f8(WS * g2[:, None] * Cq), "wck": _f8(WS * Ck),
        "wcv": _f8(WS * Cv),
        "wmp": _f8(WS * Wmp), "wcp": _f8(WS * Wcp),
        "wf1": W1.astype(BFH), "wf2": inp["f_w2"].astype(BFH),
        "bsq": (WS * bq).astype(np.float32),
        "bsk": (WS * bk).astype(np.float32),
        "bcq": (WS * bcq).astype(np.float32),
        "bck": (WS * bck).astype(np.float32),
        "bcp": bcp.astype(np.float32),
        "bf1": bf1.astype(np.float32),
        "bf2": inp["f_b2"].astype(np.float32),
    }

    x = inp["x"]
    xe = inp["x_e"]
    in_maps = []
    for core in range(NCORE):
        b, j = core // 4, core % 4
        own = j + 4 * np.arange(128)
        rest = np.setdiff1d(np.arange(512), own)
        rel = np.concatenate([own, rest])              # within-chunk order
        perm = np.concatenate([c * 512 + rel for c in range(4)])
        # diag mask: position p=128*kb+s_par in a chunk is visible to own
        # row q_i of the same chunk iff rel[p] <= j + 4*q_i
        s_pos = rel.reshape(4, 128)                    # [kb, s_par]
        qg = j + 4 * np.arange(128)                    # [q_i]
        dm = (s_pos[:, :, None] <= qg[None, None, :])  # [kb, s_par, q]
        dm = np.transpose(dm, (1, 0, 2)).astype(np.float32)  # [s_par, kb, q]
        dm = np.repeat(dm[:, :, None, :], 2, axis=2)   # [s_par, kb, 2, q]
        m = dict(shared)
        m["xT"] = _bf(x[b][perm].T)
        m["xoT"] = _bf((x[b][j + 4 * np.arange(512)] + bmp).T)
        m["xeT"] = _f8(np.ascontiguousarray(xe[b].T))
        m["dmask"] = np.ascontiguousarray(dm).astype(F8H)
        in_maps.append(m)

    res = run_bass_kernel_spmd(nc, in_maps, core_ids=list(range(NCORE)))
    out = np.empty((B, T, C), np.float32)
    for core in range(NCORE):
        b, j = core // 4, core % 4
        out[b, j + 4 * np.arange(512), :] = res.results[core]["outT"].T
    return out
